# revision 14
# baseline (speedup 1.0000x reference)
"""EvolveGCN Trainium2 kernel (8-core SPMD).

Strategy:
  - Nodes are sharded by destination across the 8 cores (6250 nodes each).
  - The GRU weight evolution (tiny 128x128 mats) is computed on the host.
  - Per (layer, t): each core computes H = x @ Q for its node slice,
    AllGathers the bf16 table into HBM, gathers its edges' source rows via
    dma_gather (int16 indices -> table split into two halves), and
    aggregates messages with PE matmuls against on-device-built one-hot
    selection matrices (edge-weight folded in).
  - Host bin-packs destination nodes into fixed-capacity "slots" (64 node
    positions, 4+4 chunks of 128 edges per table-half) so the device
    program is fully static; padding edges have w=0.
"""

import sys

for _p in ("/opt/trn_rl_repo", "/opt/pypackages"):
    if _p not in sys.path:
        sys.path.append(_p)

from dataclasses import dataclass

import numpy as np
import ml_dtypes

BF16 = ml_dtypes.bfloat16
NEG_SLOPE = (1.0 / 8.0 + 1.0 / 3.0) / 2.0


@dataclass(frozen=True)
class Cfg:
    T: int = 6
    N: int = 50000
    F: int = 128
    L: int = 2
    NCORES: int = 8
    POS: int = 64         # node positions per slot
    CA: int = 4           # chunks (of 128 edges) per slot for table half A
    CB: int = 4
    SLOTS: int = 16       # slots per psum pass
    NPASS: int = 7

    @property
    def NPC(self):  # nodes per core (dst sharding)
        return self.N // self.NCORES

    @property
    def NSLOT(self):
        return self.SLOTS * self.NPASS

    @property
    def POS_TOT(self):  # padded positions per core
        return self.NSLOT * self.POS

    @property
    def PASS_W(self):  # psum width per pass
        return self.SLOTS * self.POS

    @property
    def CHT(self):  # chunks per pass (A block + B block)
        return (self.CA + self.CB) * self.SLOTS

    @property
    def HALF1(self):  # layer-1 gather table half size
        return self.N // 2

    @property
    def HALF2(self):  # layer-2 table half size (= cores 0..NC/2-1)
        return self.NCORES * self.POS_TOT // 2

    @property
    def XJ(self):  # 128-chunks for the x@Q matmul of one core slice
        return (self.NPC + 127) // 128

    @property
    def HJ(self):  # 128-chunks per pass for the H2 matmul
        return (self.PASS_W + 127) // 128


CFG = Cfg()


# ----------------------------------------------------------------- host math

def host_gru(gate_W, gate_U, gate_b, W0, T):
    """Evolve the GCN weight through the GRU on the host.

    Returns Q[l, t] float32 [L, T, F, F]."""
    L = gate_W.shape[0]
    F = W0.shape[-1]
    out = np.zeros((L, T, F, F), dtype=np.float32)

    def sigmoid(v):
        return 1.0 / (1.0 + np.exp(-v))

    for l in range(L):
        Q = W0[l].astype(np.float32)
        gW, gU, gb = (np.asarray(a[l], dtype=np.float32) for a in (gate_W, gate_U, gate_b))
        for t in range(T):
            z = sigmoid(gW[0] @ Q + gU[0] @ Q + gb[0])
            r = sigmoid(gW[1] @ Q + gU[1] @ Q + gb[1])
            h = np.tanh(gW[2] @ Q + gU[2] @ (r * Q) + gb[2])
            Q = (1.0 - z) * Q + z * h
            out[l, t] = Q
    return out


def pack_core_t(dst_local, col, w, cfg: Cfg):
    """Bin-pack one core's edges at one timestep into the static slot layout.

    Returns (pos, perm, pad_a, pad_b) where
      pos:  [NPC] position assigned to each local node
      perm: per (pass, half) -> int array of edge ids (-1 for padding) laid
            out as [CH*128] in chunk order (slot-major).
    """
    E = dst_local.shape[0]
    half = (col >= cfg.HALF1).astype(np.int8)  # 0 = A, 1 = B

    # per-node degree split by half
    degA = np.bincount(dst_local[half == 0], minlength=cfg.NPC)
    degB = np.bincount(dst_local[half == 1], minlength=cfg.NPC)

    capA, capB = cfg.CA * 128, cfg.CB * 128
    remA = np.full(cfg.NSLOT, capA, dtype=np.int64)
    remB = np.full(cfg.NSLOT, capB, dtype=np.int64)
    remN = np.full(cfg.NSLOT, cfg.POS, dtype=np.int64)

    order = np.argsort(-(degA + degB), kind="stable")
    slot_of = np.empty(cfg.NPC, dtype=np.int64)
    pos_in_slot = np.empty(cfg.NPC, dtype=np.int64)
    for n in order:
        da, db = degA[n], degB[n]
        ok = np.flatnonzero((remA >= da) & (remB >= db) & (remN > 0))
        if ok.size == 0:
            raise RuntimeError("bin packing failed; increase capacity")
        s = ok[0]
        slot_of[n] = s
        pos_in_slot[n] = cfg.POS - remN[s]
        remA[s] -= da
        remB[s] -= db
        remN[s] -= 1
    pos = slot_of * cfg.POS + pos_in_slot

    # assign edges to chunk lanes: order edges by (slot, half)
    eslot = slot_of[dst_local]
    key = eslot * 2 + half
    eorder = np.argsort(key, kind="stable")
    counts = np.bincount(key, minlength=cfg.NSLOT * 2)
    starts = np.concatenate(([0], np.cumsum(counts)))

    perms = []
    for p in range(cfg.NPASS):
        for h, ch in ((0, cfg.CA), (1, cfg.CB)):
            blk = np.full(cfg.SLOTS * ch * 128, -1, dtype=np.int64)
            for si in range(cfg.SLOTS):
                s = p * cfg.SLOTS + si
                k = s * 2 + h
                seg = eorder[starts[k]:starts[k + 1]]
                assert seg.size <= ch * 128
                blk[si * ch * 128: si * ch * 128 + seg.size] = seg
            perms.append(blk)
    return pos, perms


def build_edge_arrays(dst_local, col, w, pos_global_fn, pos, perms, cfg: Cfg):
    """Build per-(core,t) device arrays from a packing.

    pos_global_fn(col_array) -> layer-2 table positions for original col ids.
    Returns dict with idx1/idx2 [NPASS,2,128,CH*8] int16, dstl/w [128, NPASS*CHT] bf16.
    """
    col2 = pos_global_fn(col)
    dstl_full = np.zeros((128, cfg.NPASS * cfg.CHT), dtype=np.float32)
    w_full = np.zeros((128, cfg.NPASS * cfg.CHT), dtype=np.float32)
    idx1_out = []
    idx2_out = []

    ci = 0  # global chunk index in [0, NPASS*CHT)
    for p in range(cfg.NPASS):
        for bi, (h, ch) in enumerate(((0, cfg.CA), (1, cfg.CB))):
            blk = perms[p * 2 + bi]
            nch = cfg.SLOTS * ch
            e = blk.reshape(nch, 128)
            valid = e >= 0
            esafe = np.where(valid, e, 0)

            # gather indices within table half
            c_orig = col[esafe]
            c_l2 = col2[esafe]
            i1 = np.where(valid, c_orig - h * cfg.HALF1, 0).astype(np.int64)
            i2 = np.where(valid, c_l2 - h * cfg.HALF2, 0).astype(np.int64)
            assert i1.min() >= 0 and i1.max() < cfg.HALF1
            assert i2.min() >= 0 and i2.max() < cfg.HALF2

            def wrap(ix):
                # ix [nch, 128] in chunk order -> flat order i -> tile[i%16, i//16]
                flat = ix.reshape(-1)
                tile = flat.reshape(-1, 16).T.astype(np.int16)  # [16, nch*8]
                return np.tile(tile, (8, 1))  # replicate to 128 partitions

            idx1_out.append(wrap(i1))
            idx2_out.append(wrap(i2))

            # dstl / w laid out [partition=lane, chunk]
            dl = np.where(valid, pos[dst_local[esafe]] % cfg.POS, 0)
            wv = np.where(valid, w[esafe], 0.0)
            dstl_full[:, ci:ci + nch] = dl.T
            w_full[:, ci:ci + nch] = wv.T
            ci += nch

    # group idx arrays per pass: [NPASS, 2, 128, max_cols]; A and B blocks may
    # have different widths when CA != CB -- keep them separate.
    return {
        "idx1": idx1_out,  # list of [128, nch*8] int16, order (pass, half)
        "idx2": idx2_out,
        "dstl": dstl_full.astype(BF16),
        "w": w_full.astype(BF16),
    }


def host_preprocess(x, edge_index, edge_weight, gate_W, gate_U, gate_b, W0, cfg: Cfg):
    """Build all per-core device inputs. Returns (in_maps, meta)."""
    T, N, F = x.shape
    q = host_gru(gate_W, gate_U, gate_b, W0, T)  # [L,T,F,F] f32

    dst = np.asarray(edge_index[:, 0], dtype=np.int64)
    col = np.asarray(edge_index[:, 1], dtype=np.int64)
    w = np.asarray(edge_weight, dtype=np.float32)

    owner = dst // cfg.NPC

    # phase 1: pack every (core, t); collect pos maps
    pos_all = np.zeros((T, N), dtype=np.int64)  # position of node in its owner's layout
    packs = {}
    for t in range(T):
        for c in range(cfg.NCORES):
            m = owner[t] == c
            dl = dst[t][m] - c * cfg.NPC
            pos, perms = pack_core_t(dl, col[t][m], w[t][m], cfg)
            packs[(t, c)] = (dl, col[t][m], w[t][m], pos, perms)
            pos_all[t, c * cfg.NPC: (c + 1) * cfg.NPC] = pos

    # phase 2: per-core arrays
    iota = np.broadcast_to(np.arange(cfg.POS, dtype=np.float32), (128, cfg.POS)).astype(BF16)
    qbf = q.astype(BF16)  # [L,T,F,F]

    in_maps = []
    for c in range(cfg.NCORES):
        x1T = np.ascontiguousarray(
            np.transpose(x[:, c * cfg.NPC:(c + 1) * cfg.NPC, :], (0, 2, 1))
        ).astype(BF16)  # [T, F, NPC]

        idx1_l, idx2_l, dstl_l, w_l = [], [], [], []
        for t in range(T):
            dl, ct, wt, pos, perms = packs[(t, c)]

            def pos_global(carr, t=t):
                own = carr // cfg.NPC
                return own * cfg.POS_TOT + pos_all[t, carr]

            arrs = build_edge_arrays(dl, ct, wt, pos_global, pos, perms, cfg)
            idx1_l.append(arrs["idx1"])
            idx2_l.append(arrs["idx2"])
            dstl_l.append(arrs["dstl"])
            w_l.append(arrs["w"])

        # idx tensor layout: [T, NPASS, 128, colsA + colsB] (A block then B)
        colsA, colsB = cfg.CA * cfg.SLOTS * 8, cfg.CB * cfg.SLOTS * 8
        idx1 = np.zeros((T, cfg.NPASS, 128, colsA + colsB), dtype=np.int16)
        idx2 = np.zeros_like(idx1)
        for t in range(T):
            for p in range(cfg.NPASS):
                idx1[t, p, :, :colsA] = idx1_l[t][p * 2]
                idx1[t, p, :, colsA:] = idx1_l[t][p * 2 + 1]
                idx2[t, p, :, :colsA] = idx2_l[t][p * 2]
                idx2[t, p, :, colsA:] = idx2_l[t][p * 2 + 1]

        in_maps.append({
            "x1T": x1T,
            "qmat": qbf,
            "iota": iota,
            "idx1": idx1,
            "idx2": idx2,
            "dstl": np.stack(dstl_l),  # [T, 128, NPASS*CHT]
            "warr": np.stack(w_l),
        })

    meta = {"pos_all": pos_all}
    return in_maps, meta


def host_assemble(results, pos_all, cfg: Cfg):
    """results: per-core dicts with 'out' [T,128,POS_TOT] f32 -> [T,N,F] f32."""
    T, N = pos_all.shape
    out = np.zeros((T, N, cfg.F), dtype=np.float32)
    for c, r in enumerate(results):
        dev = r["out"]  # [T, F, POS_TOT]
        for t in range(T):
            p = pos_all[t, c * cfg.NPC:(c + 1) * cfg.NPC]
            out[t, c * cfg.NPC:(c + 1) * cfg.NPC, :] = dev[t][:, p].T
    return out


# ------------------------------------------------------------- bass program

def build_bass(cfg: Cfg):
    import concourse.mybir as mybir
    import concourse.tile as tile
    from concourse.bacc import Bacc

    fp32 = mybir.dt.float32
    bf16 = mybir.dt.bfloat16
    i16 = mybir.dt.int16
    AF = mybir.ActivationFunctionType

    nc = Bacc()
    T, F = cfg.T, cfg.F
    colsA, colsB = cfg.CA * cfg.SLOTS * 8, cfg.CB * cfg.SLOTS * 8

    x1T = nc.declare_dram_parameter("x1T", [T, F, cfg.NPC], bf16, isOutput=False)
    qmat = nc.declare_dram_parameter("qmat", [cfg.L, T, F, F], bf16, isOutput=False)
    iota_in = nc.declare_dram_parameter("iota", [128, cfg.POS], bf16, isOutput=False)
    idx1 = nc.declare_dram_parameter("idx1", [T, cfg.NPASS, 128, colsA + colsB], i16, isOutput=False)
    idx2 = nc.declare_dram_parameter("idx2", [T, cfg.NPASS, 128, colsA + colsB], i16, isOutput=False)
    dstl_in = nc.declare_dram_parameter("dstl", [T, 128, cfg.NPASS * cfg.CHT], bf16, isOutput=False)
    w_in = nc.declare_dram_parameter("warr", [T, 128, cfg.NPASS * cfg.CHT], bf16, isOutput=False)
    out_d = nc.declare_dram_parameter("out", [T, F, cfg.POS_TOT], fp32, isOutput=True)

    # per-timestep scratch tables: written once, read once -> no WAR reuse
    t1own = nc.dram_tensor("t1own", [T, cfg.NPC, F], bf16)
    t2own = nc.dram_tensor("t2own", [T, cfg.POS_TOT, F], bf16)
    if cfg.NCORES > 1:
        # Shared scratch outputs are only supported for >4-core groups
        kw = {"addr_space": "Shared"} if cfg.NCORES > 4 else {}
        t1full = nc.dram_tensor("t1full", [T, cfg.N, F], bf16, **kw)
        t2full = nc.dram_tensor("t2full", [T, cfg.NCORES * cfg.POS_TOT, F], bf16, **kw)
    else:
        t1full, t2full = t1own, t2own
    groups = [list(range(cfg.NCORES))]

    KB = 8  # chunks per S-build instruction

    with tile.TileContext(nc) as tc:
        with (
            tc.tile_pool(name="const", bufs=1) as constp,
            tc.tile_pool(name="xin", bufs=2) as xinp,
            tc.tile_pool(name="stage", bufs=2) as stagep,
            tc.tile_pool(name="msg", bufs=2) as msgp,
            tc.tile_pool(name="meta", bufs=2) as metap,
            tc.tile_pool(name="sbuf", bufs=2) as sp,
            tc.tile_pool(name="spsum", bufs=2, space="PSUM") as psp,
            tc.tile_pool(name="hpsum", bufs=4, space="PSUM") as hps,
        ):
            iota_t = constp.tile([128, cfg.POS], bf16)
            nc.sync.dma_start(out=iota_t[:], in_=iota_in[:, :])
            q_t = constp.tile([128, cfg.L * T * F], bf16)
            for l in range(cfg.L):
                for t in range(T):
                    o = (l * T + t) * F
                    nc.sync.dma_start(out=q_t[:, o:o + F], in_=qmat[l, t, :, :])

            def qsl(l, t):
                o = (l * T + t) * F
                return q_t[:, o:o + F]

            def table_matmul_store(src_sb, nrows, qap, dest_dram):
                """src_sb [128, nrows] bf16 (lhsT layout) -> dest rows [nrows, F]."""
                nj = (nrows + 127) // 128
                stage = stagep.tile([128, nj * F], bf16, tag="tstage")
                for j0 in range(0, nj, 4):
                    jn = min(4, nj - j0)
                    ps = hps.tile([128, 4 * F], fp32, tag="hps")
                    for j in range(j0, j0 + jn):
                        m = min(128, nrows - j * 128)
                        nc.tensor.matmul(
                            out=ps[:m, (j - j0) * F:(j - j0 + 1) * F],
                            lhsT=src_sb[:, j * 128:j * 128 + m],
                            rhs=qap,
                            start=True, stop=True,
                        )
                    nc.scalar.activation(
                        out=stage[:, j0 * F:(j0 + jn) * F],
                        in_=ps[:, :jn * F],
                        func=AF.Copy,
                    )
                nfull = nrows // 128
                if nfull:
                    nc.sync.dma_start(
                        out=dest_dram[0:nfull * 128, :].rearrange("(j p) f -> p j f", p=128),
                        in_=stage[:, :nfull * F].rearrange("p (j f) -> p j f", j=nfull),
                    )
                rem = nrows - nfull * 128
                if rem:
                    nc.sync.dma_start(
                        out=dest_dram[nfull * 128:nrows, :],
                        in_=stage[:rem, nfull * F:(nfull + 1) * F],
                    )

            for t in range(T):
                # ---- layer-1 table: H1 = x1 @ Q1 for our node slice
                x1t = xinp.tile([128, cfg.NPC], bf16, tag="x1t")
                nc.sync.dma_start(out=x1t[:], in_=x1T[t, :, :])
                table_matmul_store(x1t, cfg.NPC, qsl(0, t), t1own[t, :, :])
                if cfg.NCORES > 1:
                    nc.gpsimd.collective_compute(
                        "AllGather", mybir.AluOpType.bypass,
                        replica_groups=groups,
                        ins=[t1own[t, :, :]], outs=[t1full[t, :, :]],
                    )

                for l in range(cfg.L):
                    table = t1full[t] if l == 0 else t2full[t]
                    halfsz = cfg.HALF1 if l == 0 else cfg.HALF2
                    idx = idx1 if l == 0 else idx2
                    for p in range(cfg.NPASS):
                        msg = msgp.tile([128, cfg.CHT, F], bf16, tag="msg")
                        idxt = metap.tile([128, colsA + colsB], i16, tag="idx")
                        nc.sync.dma_start(out=idxt[:], in_=idx[t, p, :, :])
                        for h, ch, c0, ic0 in (
                            (0, cfg.CA * cfg.SLOTS, 0, 0),
                            (1, cfg.CB * cfg.SLOTS, cfg.CA * cfg.SLOTS, colsA),
                        ):
                            nc.gpsimd.dma_gather(
                                msg[:, c0:c0 + ch, :],
                                table[h * halfsz:(h + 1) * halfsz, :],
                                idxt[:, ic0:ic0 + ch * 8],
                                num_idxs=ch * 128,
                                num_idxs_reg=ch * 128,
                                elem_size=F,
                                single_packet=False,
                            )

                        dstlt = metap.tile([128, cfg.CHT], bf16, tag="dstl")
                        wt = metap.tile([128, cfg.CHT], bf16, tag="wt")
                        nc.sync.dma_start(out=dstlt[:], in_=dstl_in[t, :, p * cfg.CHT:(p + 1) * cfg.CHT])
                        nc.sync.dma_start(out=wt[:], in_=w_in[t, :, p * cfg.CHT:(p + 1) * cfg.CHT])

                        S = sp.tile([128, cfg.CHT * cfg.POS], bf16, tag="S")
                        for b0 in range(0, cfg.CHT, KB):
                            bn = min(KB, cfg.CHT - b0)
                            ssl = S[:, b0 * cfg.POS:(b0 + bn) * cfg.POS]
                            nc.vector.tensor_tensor(
                                out=ssl,
                                in0=dstlt[:, b0:b0 + bn].unsqueeze(-1).broadcast_to([128, bn, cfg.POS]),
                                in1=iota_t[:].unsqueeze(1).broadcast_to([128, bn, cfg.POS]),
                                op=mybir.AluOpType.is_equal,
                            )
                            nc.vector.tensor_tensor(
                                out=ssl,
                                in0=ssl,
                                in1=wt[:, b0:b0 + bn].unsqueeze(-1).broadcast_to([128, bn, cfg.POS]),
                                op=mybir.AluOpType.mult,
                            )

                        aggp = psp.tile([128, cfg.PASS_W], fp32, tag="agg")
                        for si in range(cfg.SLOTS):
                            cids = (
                                [si * cfg.CA + k for k in range(cfg.CA)]
                                + [cfg.CA * cfg.SLOTS + si * cfg.CB + k for k in range(cfg.CB)]
                            )
                            for ki, ci in enumerate(cids):
                                nc.tensor.matmul(
                                    out=aggp[:, si * cfg.POS:(si + 1) * cfg.POS],
                                    lhsT=msg[:, ci, :],
                                    rhs=S[:, ci * cfg.POS:(ci + 1) * cfg.POS],
                                    start=(ki == 0),
                                    stop=(ki == len(cids) - 1),
                                )

                        # leaky relu: max(x, slope * x) (slope in (0,1))
                        lk = sp.tile([128, cfg.PASS_W], fp32, tag="lk")
                        nc.scalar.activation(out=lk[:], in_=aggp[:], func=AF.Copy, scale=NEG_SLOPE)
                        if l == 0:
                            x2t = sp.tile([128, cfg.PASS_W], bf16, tag="x2t")
                            nc.vector.tensor_tensor(
                                out=x2t[:], in0=aggp[:], in1=lk[:], op=mybir.AluOpType.max,
                            )
                            table_matmul_store(
                                x2t, cfg.PASS_W, qsl(1, t),
                                t2own[t, p * cfg.PASS_W:(p + 1) * cfg.PASS_W, :],
                            )
                        else:
                            outt = sp.tile([128, cfg.PASS_W], fp32, tag="outt")
                            nc.vector.tensor_tensor(
                                out=outt[:], in0=aggp[:], in1=lk[:], op=mybir.AluOpType.max,
                            )
                            nc.sync.dma_start(
                                out=out_d[t, :, p * cfg.PASS_W:(p + 1) * cfg.PASS_W],
                                in_=outt[:],
                            )
                    if l == 0 and cfg.NCORES > 1:
                        nc.gpsimd.collective_compute(
                            "AllGather", mybir.AluOpType.bypass,
                            replica_groups=groups,
                            ins=[t2own[t, :, :]], outs=[t2full[t, :, :]],
                        )
    nc.finalize()
    return nc


# ------------------------------------------------------------------- driver

TRACE = False
LAST_RESULT = None


def kernel(x, edge_index, edge_weight, gate_W, gate_U, gate_b, W0):
    global LAST_RESULT
    from concourse.bass_utils import run_bass_kernel_spmd

    cfg = CFG
    x = np.asarray(x)
    in_maps, meta = host_preprocess(
        x, np.asarray(edge_index), np.asarray(edge_weight),
        np.asarray(gate_W), np.asarray(gate_U), np.asarray(gate_b),
        np.asarray(W0), cfg,
    )
    nc = build_bass(cfg)
    res = run_bass_kernel_spmd(nc, in_maps, list(range(cfg.NCORES)), trace=TRACE)
    LAST_RESULT = res
    return host_assemble(res.results, meta["pos_all"], cfg).astype(np.float32)


# revision 15
# speedup vs baseline: 2.3292x; 2.3292x over previous
"""EvolveGCN Trainium2 kernel (8-core SPMD).

Strategy:
  - Nodes are sharded by destination across the 8 cores (6250 nodes each).
  - The GRU weight evolution (tiny 128x128 mats) is computed on the host.
  - Per (layer, t): each core computes H = x @ Q for its node slice,
    AllGathers the bf16 table into HBM, gathers its edges' source rows via
    dma_gather (int16 indices -> table split into two halves), and
    aggregates messages with PE matmuls against on-device-built one-hot
    selection matrices (edge-weight folded in).
  - Host bin-packs destination nodes into fixed-capacity "slots" (64 node
    positions, 4+4 chunks of 128 edges per table-half) so the device
    program is fully static; padding edges have w=0.
"""

import sys

for _p in ("/opt/trn_rl_repo", "/opt/pypackages"):
    if _p not in sys.path:
        sys.path.append(_p)

from dataclasses import dataclass

import numpy as np
import ml_dtypes

BF16 = ml_dtypes.bfloat16
NEG_SLOPE = (1.0 / 8.0 + 1.0 / 3.0) / 2.0


@dataclass(frozen=True)
class Cfg:
    T: int = 6
    N: int = 50000
    F: int = 128
    L: int = 2
    NCORES: int = 8
    POS: int = 64         # node positions per slot
    CA: int = 4           # chunks (of 128 edges) per slot for table half A
    CB: int = 4
    SLOTS: int = 16       # slots per psum pass
    NPASS: int = 7

    @property
    def NPC(self):  # nodes per core (dst sharding)
        return self.N // self.NCORES

    @property
    def NSLOT(self):
        return self.SLOTS * self.NPASS

    @property
    def POS_TOT(self):  # padded positions per core
        return self.NSLOT * self.POS

    @property
    def PASS_W(self):  # psum width per pass
        return self.SLOTS * self.POS

    @property
    def CHT(self):  # chunks per pass (A block + B block)
        return (self.CA + self.CB) * self.SLOTS

    @property
    def HALF1(self):  # layer-1 gather table half size
        return self.N // 2

    @property
    def HALF2(self):  # layer-2 table half size (= cores 0..NC/2-1)
        return self.NCORES * self.POS_TOT // 2

    @property
    def XJ(self):  # 128-chunks for the x@Q matmul of one core slice
        return (self.NPC + 127) // 128

    @property
    def HJ(self):  # 128-chunks per pass for the H2 matmul
        return (self.PASS_W + 127) // 128


CFG = Cfg()


# ----------------------------------------------------------------- host math

def host_gru(gate_W, gate_U, gate_b, W0, T):
    """Evolve the GCN weight through the GRU on the host.

    Returns Q[l, t] float32 [L, T, F, F]."""
    L = gate_W.shape[0]
    F = W0.shape[-1]
    out = np.zeros((L, T, F, F), dtype=np.float32)

    def sigmoid(v):
        return 1.0 / (1.0 + np.exp(-v))

    for l in range(L):
        Q = W0[l].astype(np.float32)
        gW, gU, gb = (np.asarray(a[l], dtype=np.float32) for a in (gate_W, gate_U, gate_b))
        for t in range(T):
            z = sigmoid(gW[0] @ Q + gU[0] @ Q + gb[0])
            r = sigmoid(gW[1] @ Q + gU[1] @ Q + gb[1])
            h = np.tanh(gW[2] @ Q + gU[2] @ (r * Q) + gb[2])
            Q = (1.0 - z) * Q + z * h
            out[l, t] = Q
    return out


def pack_core_t(dst_local, col, w, cfg: Cfg):
    """Bin-pack one core's edges at one timestep into the static slot layout.

    Returns (pos, perm, pad_a, pad_b) where
      pos:  [NPC] position assigned to each local node
      perm: per (pass, half) -> int array of edge ids (-1 for padding) laid
            out as [CH*128] in chunk order (slot-major).
    """
    E = dst_local.shape[0]
    half = (col >= cfg.HALF1).astype(np.int8)  # 0 = A, 1 = B

    # per-node degree split by half
    degA = np.bincount(dst_local[half == 0], minlength=cfg.NPC)
    degB = np.bincount(dst_local[half == 1], minlength=cfg.NPC)

    capA, capB = cfg.CA * 128, cfg.CB * 128
    remA = np.full(cfg.NSLOT, capA, dtype=np.int64)
    remB = np.full(cfg.NSLOT, capB, dtype=np.int64)
    remN = np.full(cfg.NSLOT, cfg.POS, dtype=np.int64)

    order = np.argsort(-(degA + degB), kind="stable")
    slot_of = np.empty(cfg.NPC, dtype=np.int64)
    pos_in_slot = np.empty(cfg.NPC, dtype=np.int64)
    for n in order:
        da, db = degA[n], degB[n]
        ok = np.flatnonzero((remA >= da) & (remB >= db) & (remN > 0))
        if ok.size == 0:
            raise RuntimeError("bin packing failed; increase capacity")
        s = ok[0]
        slot_of[n] = s
        pos_in_slot[n] = cfg.POS - remN[s]
        remA[s] -= da
        remB[s] -= db
        remN[s] -= 1
    pos = slot_of * cfg.POS + pos_in_slot

    # assign edges to chunk lanes: order edges by (slot, half)
    eslot = slot_of[dst_local]
    key = eslot * 2 + half
    eorder = np.argsort(key, kind="stable")
    counts = np.bincount(key, minlength=cfg.NSLOT * 2)
    starts = np.concatenate(([0], np.cumsum(counts)))

    perms = []
    for p in range(cfg.NPASS):
        for h, ch in ((0, cfg.CA), (1, cfg.CB)):
            blk = np.full(cfg.SLOTS * ch * 128, -1, dtype=np.int64)
            for si in range(cfg.SLOTS):
                s = p * cfg.SLOTS + si
                k = s * 2 + h
                seg = eorder[starts[k]:starts[k + 1]]
                assert seg.size <= ch * 128
                blk[si * ch * 128: si * ch * 128 + seg.size] = seg
            perms.append(blk)
    return pos, perms


def build_edge_arrays(dst_local, col, w, pos_global_fn, pos, perms, cfg: Cfg):
    """Build per-(core,t) device arrays from a packing.

    pos_global_fn(col_array) -> layer-2 table positions for original col ids.
    Returns dict with idx1/idx2 [NPASS,2,128,CH*8] int16, dstl/w [128, NPASS*CHT] bf16.
    """
    col2 = pos_global_fn(col)
    dstl_full = np.zeros((128, cfg.NPASS * cfg.CHT), dtype=np.float32)
    w_full = np.zeros((128, cfg.NPASS * cfg.CHT), dtype=np.float32)
    idx1_out = []
    idx2_out = []

    ci = 0  # global chunk index in [0, NPASS*CHT)
    for p in range(cfg.NPASS):
        for bi, (h, ch) in enumerate(((0, cfg.CA), (1, cfg.CB))):
            blk = perms[p * 2 + bi]
            nch = cfg.SLOTS * ch
            e = blk.reshape(nch, 128)
            valid = e >= 0
            esafe = np.where(valid, e, 0)

            # gather indices within table half
            c_orig = col[esafe]
            c_l2 = col2[esafe]
            i1 = np.where(valid, c_orig - h * cfg.HALF1, 0).astype(np.int64)
            i2 = np.where(valid, c_l2 - h * cfg.HALF2, 0).astype(np.int64)
            assert i1.min() >= 0 and i1.max() < cfg.HALF1
            assert i2.min() >= 0 and i2.max() < cfg.HALF2

            def wrap(ix):
                # ix [nch, 128] in chunk order -> flat order i -> tile[i%16, i//16]
                flat = ix.reshape(-1)
                tile = flat.reshape(-1, 16).T.astype(np.int16)  # [16, nch*8]
                return np.tile(tile, (8, 1))  # replicate to 128 partitions

            idx1_out.append(wrap(i1))
            idx2_out.append(wrap(i2))

            # dstl / w laid out [partition=lane, chunk]
            dl = np.where(valid, pos[dst_local[esafe]] % cfg.POS, 0)
            wv = np.where(valid, w[esafe], 0.0)
            dstl_full[:, ci:ci + nch] = dl.T
            w_full[:, ci:ci + nch] = wv.T
            ci += nch

    # group idx arrays per pass: [NPASS, 2, 128, max_cols]; A and B blocks may
    # have different widths when CA != CB -- keep them separate.
    return {
        "idx1": idx1_out,  # list of [128, nch*8] int16, order (pass, half)
        "idx2": idx2_out,
        "dstl": dstl_full.astype(BF16),
        "w": w_full.astype(BF16),
    }


def host_preprocess(x, edge_index, edge_weight, gate_W, gate_U, gate_b, W0, cfg: Cfg):
    """Build all per-core device inputs. Returns (in_maps, meta)."""
    T, N, F = x.shape
    q = host_gru(gate_W, gate_U, gate_b, W0, T)  # [L,T,F,F] f32

    dst = np.asarray(edge_index[:, 0], dtype=np.int64)
    col = np.asarray(edge_index[:, 1], dtype=np.int64)
    w = np.asarray(edge_weight, dtype=np.float32)

    owner = dst // cfg.NPC

    # phase 1: pack every (core, t); collect pos maps
    pos_all = np.zeros((T, N), dtype=np.int64)  # position of node in its owner's layout
    packs = {}
    for t in range(T):
        for c in range(cfg.NCORES):
            m = owner[t] == c
            dl = dst[t][m] - c * cfg.NPC
            pos, perms = pack_core_t(dl, col[t][m], w[t][m], cfg)
            packs[(t, c)] = (dl, col[t][m], w[t][m], pos, perms)
            pos_all[t, c * cfg.NPC: (c + 1) * cfg.NPC] = pos

    # phase 2: per-core arrays
    iota = np.broadcast_to(np.arange(cfg.POS, dtype=np.float32), (128, cfg.POS)).astype(BF16)
    qbf = q.astype(BF16)  # [L,T,F,F]

    in_maps = []
    for c in range(cfg.NCORES):
        x1T = np.ascontiguousarray(
            np.transpose(x[:, c * cfg.NPC:(c + 1) * cfg.NPC, :], (0, 2, 1))
        ).astype(BF16)  # [T, F, NPC]

        idx1_l, idx2_l, dstl_l, w_l = [], [], [], []
        for t in range(T):
            dl, ct, wt, pos, perms = packs[(t, c)]

            def pos_global(carr, t=t):
                own = carr // cfg.NPC
                return own * cfg.POS_TOT + pos_all[t, carr]

            arrs = build_edge_arrays(dl, ct, wt, pos_global, pos, perms, cfg)
            idx1_l.append(arrs["idx1"])
            idx2_l.append(arrs["idx2"])
            dstl_l.append(arrs["dstl"])
            w_l.append(arrs["w"])

        # idx tensor layout: [T, NPASS, 128, colsA + colsB] (A block then B)
        colsA, colsB = cfg.CA * cfg.SLOTS * 8, cfg.CB * cfg.SLOTS * 8
        idx1 = np.zeros((T, cfg.NPASS, 128, colsA + colsB), dtype=np.int16)
        idx2 = np.zeros_like(idx1)
        for t in range(T):
            for p in range(cfg.NPASS):
                idx1[t, p, :, :colsA] = idx1_l[t][p * 2]
                idx1[t, p, :, colsA:] = idx1_l[t][p * 2 + 1]
                idx2[t, p, :, :colsA] = idx2_l[t][p * 2]
                idx2[t, p, :, colsA:] = idx2_l[t][p * 2 + 1]

        in_maps.append({
            "x1T": x1T,
            "qmat": qbf,
            "iota": iota,
            "idx1": idx1,
            "idx2": idx2,
            "dstl": np.stack(dstl_l),  # [T, 128, NPASS*CHT]
            "warr": np.stack(w_l),
        })

    meta = {"pos_all": pos_all}
    return in_maps, meta


def host_assemble(results, pos_all, cfg: Cfg):
    """results: per-core dicts with 'out' [T,128,POS_TOT] f32 -> [T,N,F] f32."""
    T, N = pos_all.shape
    out = np.zeros((T, N, cfg.F), dtype=np.float32)
    for c, r in enumerate(results):
        dev = r["out"]  # [T, F, POS_TOT]
        for t in range(T):
            p = pos_all[t, c * cfg.NPC:(c + 1) * cfg.NPC]
            out[t, c * cfg.NPC:(c + 1) * cfg.NPC, :] = dev[t][:, p].T
    return out


# ------------------------------------------------------------- bass program

def build_bass(cfg: Cfg):
    import concourse.mybir as mybir
    import concourse.tile as tile
    from concourse.bacc import Bacc

    fp32 = mybir.dt.float32
    bf16 = mybir.dt.bfloat16
    i16 = mybir.dt.int16
    AF = mybir.ActivationFunctionType

    nc = Bacc(num_swdge_queues=4)
    T, F = cfg.T, cfg.F
    colsA, colsB = cfg.CA * cfg.SLOTS * 8, cfg.CB * cfg.SLOTS * 8

    x1T = nc.declare_dram_parameter("x1T", [T, F, cfg.NPC], bf16, isOutput=False)
    qmat = nc.declare_dram_parameter("qmat", [cfg.L, T, F, F], bf16, isOutput=False)
    iota_in = nc.declare_dram_parameter("iota", [128, cfg.POS], bf16, isOutput=False)
    idx1 = nc.declare_dram_parameter("idx1", [T, cfg.NPASS, 128, colsA + colsB], i16, isOutput=False)
    idx2 = nc.declare_dram_parameter("idx2", [T, cfg.NPASS, 128, colsA + colsB], i16, isOutput=False)
    dstl_in = nc.declare_dram_parameter("dstl", [T, 128, cfg.NPASS * cfg.CHT], bf16, isOutput=False)
    w_in = nc.declare_dram_parameter("warr", [T, 128, cfg.NPASS * cfg.CHT], bf16, isOutput=False)
    out_d = nc.declare_dram_parameter("out", [T, F, cfg.POS_TOT], fp32, isOutput=True)

    # per-timestep scratch tables: written once, read once -> no WAR reuse
    t1own = nc.dram_tensor("t1own", [T, cfg.NPC, F], bf16)
    t2own = nc.dram_tensor("t2own", [T, cfg.POS_TOT, F], bf16)
    if cfg.NCORES > 1:
        # Shared scratch outputs are only supported for >4-core groups
        kw = {"addr_space": "Shared"} if cfg.NCORES > 4 else {}
        t1full = nc.dram_tensor("t1full", [T, cfg.N, F], bf16, **kw)
        t2full = nc.dram_tensor("t2full", [T, cfg.NCORES * cfg.POS_TOT, F], bf16, **kw)
    else:
        t1full, t2full = t1own, t2own
    groups = [list(range(cfg.NCORES))]

    KB = 8  # chunks per S-build instruction

    with tile.TileContext(nc) as tc:
        with (
            tc.tile_pool(name="const", bufs=1) as constp,
            tc.tile_pool(name="xin", bufs=2) as xinp,
            tc.tile_pool(name="stage", bufs=2) as stagep,
            tc.tile_pool(name="msg", bufs=2) as msgp,
            tc.tile_pool(name="meta", bufs=2) as metap,
            tc.tile_pool(name="sbuf", bufs=2) as sp,
            tc.tile_pool(name="spsum", bufs=2, space="PSUM") as psp,
            tc.tile_pool(name="hpsum", bufs=4, space="PSUM") as hps,
        ):
            iota_t = constp.tile([128, cfg.POS], bf16)
            nc.sync.dma_start(out=iota_t[:], in_=iota_in[:, :])
            q_t = constp.tile([128, cfg.L * T * F], bf16)
            for l in range(cfg.L):
                for t in range(T):
                    o = (l * T + t) * F
                    nc.sync.dma_start(out=q_t[:, o:o + F], in_=qmat[l, t, :, :])

            qctr = [0]

            def qsl(l, t):
                o = (l * T + t) * F
                return q_t[:, o:o + F]

            def table_matmul_store(src_sb, nrows, qap, dest_dram):
                """src_sb [128, nrows] bf16 (lhsT layout) -> dest rows [nrows, F]."""
                nj = (nrows + 127) // 128
                stage = stagep.tile([128, nj * F], bf16, tag="tstage")
                for j0 in range(0, nj, 4):
                    jn = min(4, nj - j0)
                    ps = hps.tile([128, 4 * F], fp32, tag="hps")
                    for j in range(j0, j0 + jn):
                        m = min(128, nrows - j * 128)
                        nc.tensor.matmul(
                            out=ps[:m, (j - j0) * F:(j - j0 + 1) * F],
                            lhsT=src_sb[:, j * 128:j * 128 + m],
                            rhs=qap,
                            start=True, stop=True,
                        )
                    nc.scalar.activation(
                        out=stage[:, j0 * F:(j0 + jn) * F],
                        in_=ps[:, :jn * F],
                        func=AF.Copy,
                    )
                nfull = nrows // 128
                if nfull:
                    nc.sync.dma_start(
                        out=dest_dram[0:nfull * 128, :].rearrange("(j p) f -> p j f", p=128),
                        in_=stage[:, :nfull * F].rearrange("p (j f) -> p j f", j=nfull),
                    )
                rem = nrows - nfull * 128
                if rem:
                    nc.sync.dma_start(
                        out=dest_dram[nfull * 128:nrows, :],
                        in_=stage[:rem, nfull * F:(nfull + 1) * F],
                    )

            for t in range(T):
                # ---- layer-1 table: H1 = x1 @ Q1 for our node slice
                x1t = xinp.tile([128, cfg.NPC], bf16, tag="x1t")
                nc.sync.dma_start(out=x1t[:], in_=x1T[t, :, :])
                table_matmul_store(x1t, cfg.NPC, qsl(0, t), t1own[t, :, :])
                if cfg.NCORES > 1:
                    nc.gpsimd.collective_compute(
                        "AllGather", mybir.AluOpType.bypass,
                        replica_groups=groups,
                        ins=[t1own[t, :, :]], outs=[t1full[t, :, :]],
                    )

                for l in range(cfg.L):
                    table = t1full[t] if l == 0 else t2full[t]
                    halfsz = cfg.HALF1 if l == 0 else cfg.HALF2
                    idx = idx1 if l == 0 else idx2
                    for p in range(cfg.NPASS):
                        msg = msgp.tile([128, cfg.CHT, F], bf16, tag="msg")
                        idxt = metap.tile([128, colsA + colsB], i16, tag="idx")
                        nc.sync.dma_start(out=idxt[:], in_=idx[t, p, :, :])
                        for h, ch, c0, ic0 in (
                            (0, cfg.CA * cfg.SLOTS, 0, 0),
                            (1, cfg.CB * cfg.SLOTS, cfg.CA * cfg.SLOTS, colsA),
                        ):
                            nc.gpsimd.dma_gather(
                                msg[:, c0:c0 + ch, :],
                                table[h * halfsz:(h + 1) * halfsz, :],
                                idxt[:, ic0:ic0 + ch * 8],
                                num_idxs=ch * 128,
                                num_idxs_reg=ch * 128,
                                elem_size=F,
                                single_packet=False,
                                queue_num=qctr[0] % 4,
                            )
                            qctr[0] += 1

                        dstlt = metap.tile([128, cfg.CHT], bf16, tag="dstl")
                        wt = metap.tile([128, cfg.CHT], bf16, tag="wt")
                        nc.sync.dma_start(out=dstlt[:], in_=dstl_in[t, :, p * cfg.CHT:(p + 1) * cfg.CHT])
                        nc.sync.dma_start(out=wt[:], in_=w_in[t, :, p * cfg.CHT:(p + 1) * cfg.CHT])

                        S = sp.tile([128, cfg.CHT * cfg.POS], bf16, tag="S")
                        for b0 in range(0, cfg.CHT, KB):
                            bn = min(KB, cfg.CHT - b0)
                            ssl = S[:, b0 * cfg.POS:(b0 + bn) * cfg.POS]
                            nc.vector.tensor_tensor(
                                out=ssl,
                                in0=dstlt[:, b0:b0 + bn].unsqueeze(-1).broadcast_to([128, bn, cfg.POS]),
                                in1=iota_t[:].unsqueeze(1).broadcast_to([128, bn, cfg.POS]),
                                op=mybir.AluOpType.is_equal,
                            )
                            nc.vector.tensor_tensor(
                                out=ssl,
                                in0=ssl,
                                in1=wt[:, b0:b0 + bn].unsqueeze(-1).broadcast_to([128, bn, cfg.POS]),
                                op=mybir.AluOpType.mult,
                            )

                        aggp = psp.tile([128, cfg.PASS_W], fp32, tag="agg")
                        for si in range(cfg.SLOTS):
                            cids = (
                                [si * cfg.CA + k for k in range(cfg.CA)]
                                + [cfg.CA * cfg.SLOTS + si * cfg.CB + k for k in range(cfg.CB)]
                            )
                            for ki, ci in enumerate(cids):
                                nc.tensor.matmul(
                                    out=aggp[:, si * cfg.POS:(si + 1) * cfg.POS],
                                    lhsT=msg[:, ci, :],
                                    rhs=S[:, ci * cfg.POS:(ci + 1) * cfg.POS],
                                    start=(ki == 0),
                                    stop=(ki == len(cids) - 1),
                                )

                        # leaky relu: max(x, slope * x) (slope in (0,1))
                        lk = sp.tile([128, cfg.PASS_W], fp32, tag="lk")
                        nc.scalar.activation(out=lk[:], in_=aggp[:], func=AF.Copy, scale=NEG_SLOPE)
                        if l == 0:
                            x2t = sp.tile([128, cfg.PASS_W], bf16, tag="x2t")
                            nc.vector.tensor_tensor(
                                out=x2t[:], in0=aggp[:], in1=lk[:], op=mybir.AluOpType.max,
                            )
                            table_matmul_store(
                                x2t, cfg.PASS_W, qsl(1, t),
                                t2own[t, p * cfg.PASS_W:(p + 1) * cfg.PASS_W, :],
                            )
                        else:
                            outt = sp.tile([128, cfg.PASS_W], fp32, tag="outt")
                            nc.vector.tensor_tensor(
                                out=outt[:], in0=aggp[:], in1=lk[:], op=mybir.AluOpType.max,
                            )
                            nc.sync.dma_start(
                                out=out_d[t, :, p * cfg.PASS_W:(p + 1) * cfg.PASS_W],
                                in_=outt[:],
                            )
                    if l == 0 and cfg.NCORES > 1:
                        nc.gpsimd.collective_compute(
                            "AllGather", mybir.AluOpType.bypass,
                            replica_groups=groups,
                            ins=[t2own[t, :, :]], outs=[t2full[t, :, :]],
                        )
    nc.finalize()
    return nc


# ------------------------------------------------------------------- driver

TRACE = False
LAST_RESULT = None


def kernel(x, edge_index, edge_weight, gate_W, gate_U, gate_b, W0):
    global LAST_RESULT
    from concourse.bass_utils import run_bass_kernel_spmd

    cfg = CFG
    x = np.asarray(x)
    in_maps, meta = host_preprocess(
        x, np.asarray(edge_index), np.asarray(edge_weight),
        np.asarray(gate_W), np.asarray(gate_U), np.asarray(gate_b),
        np.asarray(W0), cfg,
    )
    nc = build_bass(cfg)
    res = run_bass_kernel_spmd(nc, in_maps, list(range(cfg.NCORES)), trace=TRACE)
    LAST_RESULT = res
    return host_assemble(res.results, meta["pos_all"], cfg).astype(np.float32)


# revision 17
# speedup vs baseline: 2.7798x; 1.1934x over previous
"""EvolveGCN Trainium2 kernel (8-core SPMD).

Strategy:
  - Nodes are sharded by destination across the 8 cores (6250 nodes each).
  - The GRU weight evolution (tiny 128x128 mats) is computed on the host.
  - Per (layer, t): each core computes H = x @ Q for its node slice,
    AllGathers the bf16 table into HBM, gathers its edges' source rows via
    dma_gather (int16 indices -> table split into two halves), and
    aggregates messages with PE matmuls against on-device-built one-hot
    selection matrices (edge-weight folded in).
  - Host bin-packs destination nodes into fixed-capacity "slots" (64 node
    positions, 4+4 chunks of 128 edges per table-half) so the device
    program is fully static; padding edges have w=0.
"""

import sys

for _p in ("/opt/trn_rl_repo", "/opt/pypackages"):
    if _p not in sys.path:
        sys.path.append(_p)

from dataclasses import dataclass

import numpy as np
import ml_dtypes

BF16 = ml_dtypes.bfloat16
NEG_SLOPE = (1.0 / 8.0 + 1.0 / 3.0) / 2.0


@dataclass(frozen=True)
class Cfg:
    T: int = 6
    N: int = 50000
    F: int = 128
    L: int = 2
    NCORES: int = 8
    POS: int = 64         # node positions per slot
    CA: int = 4           # chunks (of 128 edges) per slot for table half A
    CB: int = 4
    SLOTS: int = 16       # slots per psum pass
    NPASS: int = 7
    SFP8: bool = True     # ship S matrices from host as fp8 (else build on DVE)

    @property
    def NPC(self):  # nodes per core (dst sharding)
        return self.N // self.NCORES

    @property
    def NSLOT(self):
        return self.SLOTS * self.NPASS

    @property
    def POS_TOT(self):  # padded positions per core
        return self.NSLOT * self.POS

    @property
    def PASS_W(self):  # psum width per pass
        return self.SLOTS * self.POS

    @property
    def CHT(self):  # chunks per pass (A block + B block)
        return (self.CA + self.CB) * self.SLOTS

    @property
    def HALF1(self):  # layer-1 gather table half size
        return self.N // 2

    @property
    def HALF2(self):  # layer-2 table half size (= cores 0..NC/2-1)
        return self.NCORES * self.POS_TOT // 2

    @property
    def XJ(self):  # 128-chunks for the x@Q matmul of one core slice
        return (self.NPC + 127) // 128

    @property
    def HJ(self):  # 128-chunks per pass for the H2 matmul
        return (self.PASS_W + 127) // 128


CFG = Cfg()


# ----------------------------------------------------------------- host math

def host_gru(gate_W, gate_U, gate_b, W0, T):
    """Evolve the GCN weight through the GRU on the host.

    Returns Q[l, t] float32 [L, T, F, F]."""
    L = gate_W.shape[0]
    F = W0.shape[-1]
    out = np.zeros((L, T, F, F), dtype=np.float32)

    def sigmoid(v):
        return 1.0 / (1.0 + np.exp(-v))

    for l in range(L):
        Q = W0[l].astype(np.float32)
        gW, gU, gb = (np.asarray(a[l], dtype=np.float32) for a in (gate_W, gate_U, gate_b))
        for t in range(T):
            z = sigmoid(gW[0] @ Q + gU[0] @ Q + gb[0])
            r = sigmoid(gW[1] @ Q + gU[1] @ Q + gb[1])
            h = np.tanh(gW[2] @ Q + gU[2] @ (r * Q) + gb[2])
            Q = (1.0 - z) * Q + z * h
            out[l, t] = Q
    return out


def pack_core_t(dst_local, col, w, cfg: Cfg):
    """Bin-pack one core's edges at one timestep into the static slot layout.

    Returns (pos, perm, pad_a, pad_b) where
      pos:  [NPC] position assigned to each local node
      perm: per (pass, half) -> int array of edge ids (-1 for padding) laid
            out as [CH*128] in chunk order (slot-major).
    """
    E = dst_local.shape[0]
    half = (col >= cfg.HALF1).astype(np.int8)  # 0 = A, 1 = B

    # per-node degree split by half
    degA = np.bincount(dst_local[half == 0], minlength=cfg.NPC)
    degB = np.bincount(dst_local[half == 1], minlength=cfg.NPC)

    capA, capB = cfg.CA * 128, cfg.CB * 128
    remA = np.full(cfg.NSLOT, capA, dtype=np.int64)
    remB = np.full(cfg.NSLOT, capB, dtype=np.int64)
    remN = np.full(cfg.NSLOT, cfg.POS, dtype=np.int64)

    order = np.argsort(-(degA + degB), kind="stable")
    slot_of = np.empty(cfg.NPC, dtype=np.int64)
    pos_in_slot = np.empty(cfg.NPC, dtype=np.int64)
    for n in order:
        da, db = degA[n], degB[n]
        ok = np.flatnonzero((remA >= da) & (remB >= db) & (remN > 0))
        if ok.size == 0:
            raise RuntimeError("bin packing failed; increase capacity")
        s = ok[0]
        slot_of[n] = s
        pos_in_slot[n] = cfg.POS - remN[s]
        remA[s] -= da
        remB[s] -= db
        remN[s] -= 1
    pos = slot_of * cfg.POS + pos_in_slot

    # assign edges to chunk lanes: order edges by (slot, half)
    eslot = slot_of[dst_local]
    key = eslot * 2 + half
    eorder = np.argsort(key, kind="stable")
    counts = np.bincount(key, minlength=cfg.NSLOT * 2)
    starts = np.concatenate(([0], np.cumsum(counts)))

    perms = []
    for p in range(cfg.NPASS):
        for h, ch in ((0, cfg.CA), (1, cfg.CB)):
            blk = np.full(cfg.SLOTS * ch * 128, -1, dtype=np.int64)
            for si in range(cfg.SLOTS):
                s = p * cfg.SLOTS + si
                k = s * 2 + h
                seg = eorder[starts[k]:starts[k + 1]]
                assert seg.size <= ch * 128
                blk[si * ch * 128: si * ch * 128 + seg.size] = seg
            perms.append(blk)
    return pos, perms


def build_edge_arrays(dst_local, col, w, pos_global_fn, pos, perms, cfg: Cfg):
    """Build per-(core,t) device arrays from a packing.

    pos_global_fn(col_array) -> layer-2 table positions for original col ids.
    Returns dict with idx1/idx2 [NPASS,2,128,CH*8] int16, dstl/w [128, NPASS*CHT] bf16.
    """
    col2 = pos_global_fn(col)
    dstl_full = np.zeros((128, cfg.NPASS * cfg.CHT), dtype=np.float32)
    w_full = np.zeros((128, cfg.NPASS * cfg.CHT), dtype=np.float32)
    s_blk = np.zeros((128, cfg.NPASS * cfg.CHT, cfg.POS), dtype=np.float32)
    idx1_out = []
    idx2_out = []

    ci = 0  # global chunk index in [0, NPASS*CHT)
    for p in range(cfg.NPASS):
        for bi, (h, ch) in enumerate(((0, cfg.CA), (1, cfg.CB))):
            blk = perms[p * 2 + bi]
            nch = cfg.SLOTS * ch
            e = blk.reshape(nch, 128)
            valid = e >= 0
            esafe = np.where(valid, e, 0)

            # gather indices within table half; pads get spread-out indices
            # (identical pad indices hot-spot one HBM bank)
            c_orig = col[esafe]
            c_l2 = col2[esafe]
            spread = (np.arange(e.size, dtype=np.int64).reshape(e.shape) * 2654435761)
            i1 = np.where(valid, c_orig - h * cfg.HALF1, spread % cfg.HALF1).astype(np.int64)
            i2 = np.where(valid, c_l2 - h * cfg.HALF2, spread % cfg.HALF2).astype(np.int64)
            assert i1.min() >= 0 and i1.max() < cfg.HALF1
            assert i2.min() >= 0 and i2.max() < cfg.HALF2

            def wrap(ix):
                # ix [nch, 128] in chunk order -> flat order i -> tile[i%16, i//16]
                flat = ix.reshape(-1)
                tile = flat.reshape(-1, 16).T.astype(np.int16)  # [16, nch*8]
                return np.tile(tile, (8, 1))  # replicate to 128 partitions

            idx1_out.append(wrap(i1))
            idx2_out.append(wrap(i2))

            # dstl / w laid out [partition=lane, chunk]
            dl = np.where(valid, pos[dst_local[esafe]] % cfg.POS, 0)
            wv = np.where(valid, w[esafe], 0.0)
            dstl_full[:, ci:ci + nch] = dl.T
            w_full[:, ci:ci + nch] = wv.T
            # fp8 S block: S[lane, ci + c, dl] = w
            lanes = np.broadcast_to(np.arange(128)[None, :], e.shape)
            cs = np.broadcast_to(np.arange(nch)[:, None], e.shape)
            s_blk[lanes.ravel(), (ci + cs).ravel(), dl.ravel()] = wv.ravel()
            ci += nch

    # group idx arrays per pass: [NPASS, 2, 128, max_cols]; A and B blocks may
    # have different widths when CA != CB -- keep them separate.
    return {
        "idx1": idx1_out,  # list of [128, nch*8] int16, order (pass, half)
        "idx2": idx2_out,
        "dstl": dstl_full.astype(BF16),
        "w": w_full.astype(BF16),
        "sfp8": np.round(s_blk.reshape(128, -1) * 255.0).astype(np.uint8),
    }


def host_preprocess(x, edge_index, edge_weight, gate_W, gate_U, gate_b, W0, cfg: Cfg):
    """Build all per-core device inputs. Returns (in_maps, meta)."""
    T, N, F = x.shape
    q = host_gru(gate_W, gate_U, gate_b, W0, T)  # [L,T,F,F] f32

    dst = np.asarray(edge_index[:, 0], dtype=np.int64)
    col = np.asarray(edge_index[:, 1], dtype=np.int64)
    w = np.asarray(edge_weight, dtype=np.float32)

    owner = dst // cfg.NPC

    # phase 1: pack every (core, t); collect pos maps
    pos_all = np.zeros((T, N), dtype=np.int64)  # position of node in its owner's layout
    packs = {}
    for t in range(T):
        for c in range(cfg.NCORES):
            m = owner[t] == c
            dl = dst[t][m] - c * cfg.NPC
            pos, perms = pack_core_t(dl, col[t][m], w[t][m], cfg)
            packs[(t, c)] = (dl, col[t][m], w[t][m], pos, perms)
            pos_all[t, c * cfg.NPC: (c + 1) * cfg.NPC] = pos

    # phase 2: per-core arrays
    iota = np.broadcast_to(np.arange(cfg.POS, dtype=np.float32), (128, cfg.POS)).astype(BF16)
    qbf = q.astype(BF16)  # [L,T,F,F]

    in_maps = []
    for c in range(cfg.NCORES):
        x1T = np.ascontiguousarray(
            np.transpose(x[:, c * cfg.NPC:(c + 1) * cfg.NPC, :], (0, 2, 1))
        ).astype(BF16)  # [T, F, NPC]

        idx1_l, idx2_l, dstl_l, w_l, s_l = [], [], [], [], []
        for t in range(T):
            dl, ct, wt, pos, perms = packs[(t, c)]

            def pos_global(carr, t=t):
                own = carr // cfg.NPC
                return own * cfg.POS_TOT + pos_all[t, carr]

            arrs = build_edge_arrays(dl, ct, wt, pos_global, pos, perms, cfg)
            idx1_l.append(arrs["idx1"])
            idx2_l.append(arrs["idx2"])
            dstl_l.append(arrs["dstl"])
            w_l.append(arrs["w"])
            s_l.append(arrs["sfp8"])

        # idx tensor layout: [T, NPASS, 128, colsA + colsB] (A block then B)
        colsA, colsB = cfg.CA * cfg.SLOTS * 8, cfg.CB * cfg.SLOTS * 8
        idx1 = np.zeros((T, cfg.NPASS, 128, colsA + colsB), dtype=np.int16)
        idx2 = np.zeros_like(idx1)
        for t in range(T):
            for p in range(cfg.NPASS):
                idx1[t, p, :, :colsA] = idx1_l[t][p * 2]
                idx1[t, p, :, colsA:] = idx1_l[t][p * 2 + 1]
                idx2[t, p, :, :colsA] = idx2_l[t][p * 2]
                idx2[t, p, :, colsA:] = idx2_l[t][p * 2 + 1]

        im = {
            "x1T": x1T,
            "qmat": qbf,
            "idx1": idx1,
            "idx2": idx2,
        }
        if cfg.SFP8:
            im["sfp8"] = np.stack(s_l)  # [T, 128, NPASS*CHT*POS]
        else:
            im["iota"] = iota
            im["dstl"] = np.stack(dstl_l)  # [T, 128, NPASS*CHT]
            im["warr"] = np.stack(w_l)
        in_maps.append(im)

    meta = {"pos_all": pos_all}
    return in_maps, meta


def host_assemble(results, pos_all, cfg: Cfg):
    """results: per-core dicts with 'out' [T,128,POS_TOT] f32 -> [T,N,F] f32."""
    T, N = pos_all.shape
    out = np.zeros((T, N, cfg.F), dtype=np.float32)
    for c, r in enumerate(results):
        dev = r["out"]  # [T, F, POS_TOT]
        for t in range(T):
            p = pos_all[t, c * cfg.NPC:(c + 1) * cfg.NPC]
            out[t, c * cfg.NPC:(c + 1) * cfg.NPC, :] = dev[t][:, p].T
    return out


# ------------------------------------------------------------- bass program

def build_bass(cfg: Cfg):
    import concourse.mybir as mybir
    import concourse.tile as tile
    from concourse.bacc import Bacc

    fp32 = mybir.dt.float32
    bf16 = mybir.dt.bfloat16
    i16 = mybir.dt.int16
    AF = mybir.ActivationFunctionType

    nc = Bacc(num_swdge_queues=4)
    T, F = cfg.T, cfg.F
    colsA, colsB = cfg.CA * cfg.SLOTS * 8, cfg.CB * cfg.SLOTS * 8

    u8 = mybir.dt.uint8
    x1T = nc.declare_dram_parameter("x1T", [T, F, cfg.NPC], bf16, isOutput=False)
    qmat = nc.declare_dram_parameter("qmat", [cfg.L, T, F, F], bf16, isOutput=False)
    idx1 = nc.declare_dram_parameter("idx1", [T, cfg.NPASS, 128, colsA + colsB], i16, isOutput=False)
    idx2 = nc.declare_dram_parameter("idx2", [T, cfg.NPASS, 128, colsA + colsB], i16, isOutput=False)
    if cfg.SFP8:
        sfp8_in = nc.declare_dram_parameter(
            "sfp8", [T, 128, cfg.NPASS * cfg.CHT * cfg.POS], u8, isOutput=False)
    else:
        iota_in = nc.declare_dram_parameter("iota", [128, cfg.POS], bf16, isOutput=False)
        dstl_in = nc.declare_dram_parameter("dstl", [T, 128, cfg.NPASS * cfg.CHT], bf16, isOutput=False)
        w_in = nc.declare_dram_parameter("warr", [T, 128, cfg.NPASS * cfg.CHT], bf16, isOutput=False)
    out_d = nc.declare_dram_parameter("out", [T, F, cfg.POS_TOT], fp32, isOutput=True)

    # per-timestep scratch tables: written once, read once -> no WAR reuse
    t1own = nc.dram_tensor("t1own", [T, cfg.NPC, F], bf16)
    t2own = nc.dram_tensor("t2own", [T, cfg.POS_TOT, F], bf16)
    if cfg.NCORES > 1:
        # Shared scratch outputs are only supported for >4-core groups
        kw = {"addr_space": "Shared"} if cfg.NCORES > 4 else {}
        t1full = nc.dram_tensor("t1full", [T, cfg.N, F], bf16, **kw)
        t2full = nc.dram_tensor("t2full", [T, cfg.NCORES * cfg.POS_TOT, F], bf16, **kw)
    else:
        t1full, t2full = t1own, t2own
    groups = [list(range(cfg.NCORES))]

    KB = 8  # chunks per S-build instruction

    with tile.TileContext(nc) as tc:
        with (
            tc.tile_pool(name="const", bufs=1) as constp,
            tc.tile_pool(name="xin", bufs=2) as xinp,
            tc.tile_pool(name="stage", bufs=2) as stagep,
            tc.tile_pool(name="msg", bufs=2) as msgp,
            tc.tile_pool(name="meta", bufs=2) as metap,
            tc.tile_pool(name="sbuf", bufs=2) as sp,
            tc.tile_pool(name="spsum", bufs=2, space="PSUM") as psp,
            tc.tile_pool(name="hpsum", bufs=4, space="PSUM") as hps,
        ):
            if not cfg.SFP8:
                iota_t = constp.tile([128, cfg.POS], bf16)
                nc.sync.dma_start(out=iota_t[:], in_=iota_in[:, :])
            q_t = constp.tile([128, cfg.L * T * F], bf16)
            for l in range(cfg.L):
                for t in range(T):
                    o = (l * T + t) * F
                    nc.sync.dma_start(out=q_t[:, o:o + F], in_=qmat[l, t, :, :])

            qctr = [0]

            def qsl(l, t):
                o = (l * T + t) * F
                return q_t[:, o:o + F]

            def table_matmul_store(src_sb, nrows, qap, dest_dram):
                """src_sb [128, nrows] bf16 (lhsT layout) -> dest rows [nrows, F]."""
                nj = (nrows + 127) // 128
                stage = stagep.tile([128, nj * F], bf16, tag="tstage")
                for j0 in range(0, nj, 4):
                    jn = min(4, nj - j0)
                    ps = hps.tile([128, 4 * F], fp32, tag="hps")
                    for j in range(j0, j0 + jn):
                        m = min(128, nrows - j * 128)
                        nc.tensor.matmul(
                            out=ps[:m, (j - j0) * F:(j - j0 + 1) * F],
                            lhsT=src_sb[:, j * 128:j * 128 + m],
                            rhs=qap,
                            start=True, stop=True,
                        )
                    nc.scalar.activation(
                        out=stage[:, j0 * F:(j0 + jn) * F],
                        in_=ps[:, :jn * F],
                        func=AF.Copy,
                    )
                nfull = nrows // 128
                if nfull:
                    nc.sync.dma_start(
                        out=dest_dram[0:nfull * 128, :].rearrange("(j p) f -> p j f", p=128),
                        in_=stage[:, :nfull * F].rearrange("p (j f) -> p j f", j=nfull),
                    )
                rem = nrows - nfull * 128
                if rem:
                    nc.sync.dma_start(
                        out=dest_dram[nfull * 128:nrows, :],
                        in_=stage[:rem, nfull * F:(nfull + 1) * F],
                    )

            for t in range(T):
                # ---- layer-1 table: H1 = x1 @ Q1 for our node slice
                x1t = xinp.tile([128, cfg.NPC], bf16, tag="x1t")
                nc.sync.dma_start(out=x1t[:], in_=x1T[t, :, :])
                table_matmul_store(x1t, cfg.NPC, qsl(0, t), t1own[t, :, :])
                if cfg.NCORES > 1:
                    nc.gpsimd.collective_compute(
                        "AllGather", mybir.AluOpType.bypass,
                        replica_groups=groups,
                        ins=[t1own[t, :, :]], outs=[t1full[t, :, :]],
                    )

                for l in range(cfg.L):
                    table = t1full[t] if l == 0 else t2full[t]
                    halfsz = cfg.HALF1 if l == 0 else cfg.HALF2
                    idx = idx1 if l == 0 else idx2
                    for p in range(cfg.NPASS):
                        msg = msgp.tile([128, cfg.CHT, F], bf16, tag="msg")
                        idxt = metap.tile([128, colsA + colsB], i16, tag="idx")
                        nc.sync.dma_start(out=idxt[:], in_=idx[t, p, :, :])
                        for h, ch, c0, ic0 in (
                            (0, cfg.CA * cfg.SLOTS, 0, 0),
                            (1, cfg.CB * cfg.SLOTS, cfg.CA * cfg.SLOTS, colsA),
                        ):
                            nc.gpsimd.dma_gather(
                                msg[:, c0:c0 + ch, :],
                                table[h * halfsz:(h + 1) * halfsz, :],
                                idxt[:, ic0:ic0 + ch * 8],
                                num_idxs=ch * 128,
                                num_idxs_reg=ch * 128,
                                elem_size=F,
                                single_packet=False,
                                queue_num=qctr[0] % 4,
                            )
                            qctr[0] += 1

                        S = sp.tile([128, cfg.CHT * cfg.POS], bf16, tag="S")
                        if cfg.SFP8:
                            s8 = metap.tile([128, cfg.CHT * cfg.POS], u8, tag="s8")
                            nc.sync.dma_start(
                                out=s8[:],
                                in_=sfp8_in[t, :, p * cfg.CHT * cfg.POS:(p + 1) * cfg.CHT * cfg.POS],
                            )
                            nc.vector.tensor_scalar(
                                out=S[:], in0=s8[:], scalar1=1.0 / 255.0,
                                scalar2=None, op0=mybir.AluOpType.mult,
                            )
                        else:
                            dstlt = metap.tile([128, cfg.CHT], bf16, tag="dstl")
                            wt = metap.tile([128, cfg.CHT], bf16, tag="wt")
                            nc.sync.dma_start(out=dstlt[:], in_=dstl_in[t, :, p * cfg.CHT:(p + 1) * cfg.CHT])
                            nc.sync.dma_start(out=wt[:], in_=w_in[t, :, p * cfg.CHT:(p + 1) * cfg.CHT])
                            for b0 in range(0, cfg.CHT, KB):
                                bn = min(KB, cfg.CHT - b0)
                                ssl = S[:, b0 * cfg.POS:(b0 + bn) * cfg.POS]
                                nc.vector.tensor_tensor(
                                    out=ssl,
                                    in0=dstlt[:, b0:b0 + bn].unsqueeze(-1).broadcast_to([128, bn, cfg.POS]),
                                    in1=iota_t[:].unsqueeze(1).broadcast_to([128, bn, cfg.POS]),
                                    op=mybir.AluOpType.is_equal,
                                )
                                nc.vector.tensor_tensor(
                                    out=ssl,
                                    in0=ssl,
                                    in1=wt[:, b0:b0 + bn].unsqueeze(-1).broadcast_to([128, bn, cfg.POS]),
                                    op=mybir.AluOpType.mult,
                                )

                        aggp = psp.tile([128, cfg.PASS_W], fp32, tag="agg")
                        for si in range(cfg.SLOTS):
                            cids = (
                                [si * cfg.CA + k for k in range(cfg.CA)]
                                + [cfg.CA * cfg.SLOTS + si * cfg.CB + k for k in range(cfg.CB)]
                            )
                            for ki, ci in enumerate(cids):
                                nc.tensor.matmul(
                                    out=aggp[:, si * cfg.POS:(si + 1) * cfg.POS],
                                    lhsT=msg[:, ci, :],
                                    rhs=S[:, ci * cfg.POS:(ci + 1) * cfg.POS],
                                    start=(ki == 0),
                                    stop=(ki == len(cids) - 1),
                                )

                        # leaky relu: max(x, slope * x) (slope in (0,1))
                        lk = sp.tile([128, cfg.PASS_W], fp32, tag="lk")
                        nc.scalar.activation(out=lk[:], in_=aggp[:], func=AF.Copy, scale=NEG_SLOPE)
                        if l == 0:
                            x2t = sp.tile([128, cfg.PASS_W], bf16, tag="x2t")
                            nc.vector.tensor_tensor(
                                out=x2t[:], in0=aggp[:], in1=lk[:], op=mybir.AluOpType.max,
                            )
                            table_matmul_store(
                                x2t, cfg.PASS_W, qsl(1, t),
                                t2own[t, p * cfg.PASS_W:(p + 1) * cfg.PASS_W, :],
                            )
                        else:
                            outt = sp.tile([128, cfg.PASS_W], fp32, tag="outt")
                            nc.vector.tensor_tensor(
                                out=outt[:], in0=aggp[:], in1=lk[:], op=mybir.AluOpType.max,
                            )
                            nc.sync.dma_start(
                                out=out_d[t, :, p * cfg.PASS_W:(p + 1) * cfg.PASS_W],
                                in_=outt[:],
                            )
                    if l == 0 and cfg.NCORES > 1:
                        nc.gpsimd.collective_compute(
                            "AllGather", mybir.AluOpType.bypass,
                            replica_groups=groups,
                            ins=[t2own[t, :, :]], outs=[t2full[t, :, :]],
                        )
    nc.finalize()
    return nc


# ------------------------------------------------------------------- driver

TRACE = False
LAST_RESULT = None


def kernel(x, edge_index, edge_weight, gate_W, gate_U, gate_b, W0):
    global LAST_RESULT
    from concourse.bass_utils import run_bass_kernel_spmd

    cfg = CFG
    x = np.asarray(x)
    in_maps, meta = host_preprocess(
        x, np.asarray(edge_index), np.asarray(edge_weight),
        np.asarray(gate_W), np.asarray(gate_U), np.asarray(gate_b),
        np.asarray(W0), cfg,
    )
    nc = build_bass(cfg)
    res = run_bass_kernel_spmd(nc, in_maps, list(range(cfg.NCORES)), trace=TRACE)
    LAST_RESULT = res
    return host_assemble(res.results, meta["pos_all"], cfg).astype(np.float32)
